# revision 36
# baseline (speedup 1.0000x reference)
"""ConvMod3d (StyleGAN-style modulated 3x3x3 conv, N=4 groups) on 8 trn2 cores.

Sharding: 8 shards = 4 samples x 2 H-halves (was 2 D-halves). Each core
convolves 48 full-depth slabs of 25 h-rows (64ch x 25x48) against its
sample's modulated 64x64x27 weights, producing all 46 output D-planes x 23
h'-rows. 46 planes = 23 EVEN pairs, so the PE col-strip pairing never
degrades to a half-rate single-plane group (the D-split's odd 23rd plane
cost a full-rate group span). Style modulation/demodulation on host; the
conv (99.8% of FLOPs) on device.

Per output plane d': 27 taps, each a [Cin=64 -> Cout=64] matmul over the
flattened (h,w) slab (25x48 -> window of 1216 cols) with a shifted read
offset; invalid edge columns (w'>=46) are computed and discarded on host.

PE packing (trn2 constraints: row tiling crashes the device; alternating
contract sizes back-to-back costs 2.2x). Taps packed two-per-matmul on the
contraction dim via stacked SBUF windows:
- W[d]  = slab d (partitions 0-63) | slab d+1 (64-127): fuses the
  (kd=0,kd=1) tap pairs -> 9 contract-128 streams per output plane.
- W2[d] = slab d | slab d shifted +48 cols (one h row): fuses the
  (kd=2, kh=0/1) pairs -> 3 contract-128 streams; the 3 (kd=2,kh=2)
  taps stay contract-64 on W2's lower half.
Two output planes run concurrently on PE col strips. Matmuls in bf16
(fp32 PSUM accumulation).

Head: only the first pair's low window columns are DMA'd before the first
matmuls (a matmul's DMA wait is a coarse semaphore threshold covering
every DMA issued before it); the rest stream in behind pair-0's chunks.
"""

import time

import numpy as np
import ml_dtypes

import concourse.bacc as bacc
import concourse.bass as bass
import concourse.tile as tile
from concourse import mybir
from concourse.bass_utils import run_bass_kernel_spmd

EPS = 1e-8
N, CIN, COUT = 4, 64, 64
DHW, K = 48, 3
DOUT = DHW - K + 1          # 46 output planes per core (full depth)
H_HALF = DOUT // 2          # 23 output h'-rows per core
H_IN = H_HALF + K - 1       # 25 input h-rows per core
PLANE = H_IN * DHW          # 1200 cols per slab (25 h-rows x 48 w)
PAD_COLS = 192              # tail slack so shifted reads stay in-bounds
XS_COLS = DHW * PLANE + PAD_COLS   # 48 slabs
WCOLS = 1216                # window columns; max offset used 98+1103
PLANE_OUT = H_HALF * DHW    # 1104 computed output cols (23 h'-rows x 48)
NPAIRS = DOUT // 2          # 23 output-plane pairs
CHUNKS = [(0, 512), (512, 512), (1024, 80)]
NCORES = 8
NWBLK = 14                  # weight blocks of 64 cols (13 c128 + 1 c64)
XCUT = 640                  # first-cut columns (covers chunk 0: off<=610)

F32 = mybir.dt.float32
MM_DT = mybir.dt.bfloat16
NP_MM = np.dtype(ml_dtypes.bfloat16)

_CACHE = {}
LAST_RESULTS = None  # BassKernelResults of the most recent device run


def _build_bass():
    nc = bacc.Bacc()
    xs = nc.declare_dram_parameter("xs", [CIN, XS_COLS], MM_DT, isOutput=False)
    wt = nc.declare_dram_parameter("wt", [128, NWBLK * COUT], MM_DT, isOutput=False)
    bt = nc.declare_dram_parameter("bt", [128, 1], F32, isOutput=False)
    # output stored bf16: halves output HBM traffic (measured DMA ceiling
    # is ~270GB/s and the f32 store pushed total demand right against it);
    # rounding adds ~0.1% rel err against a 2% budget
    y = nc.declare_dram_parameter(
        "y", [NPAIRS, 128, PLANE_OUT], MM_DT, isOutput=True)

    with tile.TileContext(nc) as tc:
        with (
            tc.tile_pool(name="const", bufs=1) as cpool,
            tc.tile_pool(name="xpool", bufs=28) as xpool,
            tc.tile_pool(name="opool", bufs=3) as opool,
            tc.tile_pool(name="ppool", bufs=8, space="PSUM") as ppool,
        ):
            wtile = cpool.tile([128, NWBLK * COUT], MM_DT)
            nc.sync.dma_start(out=wtile[:, :], in_=wt[:, :])
            btile = cpool.tile([128, 1], F32)
            nc.sync.dma_start(out=btile[:, :], in_=bt[:, :])

            windows = {}

            UPSHIFT = {"w": PLANE, "w2": DHW, "w3": 1}

            def load_window(fam, p, a=0, b=WCOLS, engs=None):
                key = (fam, p)
                if key in windows:
                    xw = windows[key]
                else:
                    xw = xpool.tile([128, WCOLS], MM_DT, tag="xw", name="xw")
                    windows[key] = xw
                base = p * PLANE
                up = base + UPSHIFT[fam]
                e1, e2 = engs if engs else (nc.sync, nc.sync)
                e1.dma_start(out=xw[0:64, a:b],
                             in_=xs[:, base + a:base + b])
                if up + b <= XS_COLS:
                    # W[47]'s upper half (slab 48) doesn't exist; that
                    # window is only a derivation source (lower half)
                    e2.dma_start(out=xw[64:128, a:b],
                                 in_=xs[:, up + a:up + b])

            def ensure_pair(p):
                if p >= NPAIRS:
                    return
                for d in (2 * p, 2 * p + 1):
                    if ("w", d) not in windows:
                        load_window("w", d)
                for d in (2 * p + 2, 2 * p + 3):
                    # next pair's W windows double as sources: W2[d] is
                    # derived from W[d]'s lower half by SBUF->SBUF DMA
                    # (saves ~83GB/s of HBM reads, the headroom that
                    # previously made the W3 family a net loss), W3[d]
                    # loads from HBM. Issues spread across the three
                    # DMA-capable queues; ~600ns each would saturate a
                    # single queue at 12 issues/pair.
                    if ("w", d) not in windows:
                        load_window("w", d)
                    if ("w2", d) not in windows:
                        w2 = xpool.tile([128, WCOLS], MM_DT,
                                        tag="xw", name="xw")
                        windows[("w2", d)] = w2
                        wsrc = windows[("w", d)]
                        nc.scalar.dma_start(out=w2[0:64, 0:1202],
                                            in_=wsrc[0:64, 0:1202])
                        nc.scalar.dma_start(out=w2[64:128, 0:1106],
                                            in_=wsrc[0:64, 48:1154])
                    if ("w3", d) not in windows:
                        load_window("w3", d,
                                    engs=(nc.gpsimd, nc.gpsimd))

            # PE warm-up: a few throwaway matmuls on the (first-loaded)
            # weight tile trip the HAM clock gate to K=8/8 while the first
            # windows stream in; without them the first ~5us of real
            # matmuls run at the 1.2GHz cold clock. Scratch PSUM, never
            # read.
            wps = ppool.tile([128, 512], F32, tag="ps", name="ps")
            for _ in range(5):
                nc.tensor.matmul(
                    wps[0:64, 0:512],
                    wtile[0:128, 0:64],
                    wtile[0:128, 0:512],
                    start=True, stop=True,
                )

            # pair 0: low columns only before the first matmuls, issue
            # spread across the three DMA-capable queues — each
            # DMA_DIRECT2D issue costs ~600ns and the queues run them
            # serially, so single-queue issue alone costs ~5us of head
            p0wins = [("w", 0), ("w", 1), ("w2", 2), ("w2", 3),
                      ("w3", 2), ("w3", 3)]
            issue_engs = [(nc.scalar, nc.gpsimd), (nc.sync, nc.scalar),
                          (nc.gpsimd, nc.sync), (nc.scalar, nc.gpsimd),
                          (nc.sync, nc.gpsimd), (nc.scalar, nc.sync)]
            for (fam, p), engs in zip(p0wins, issue_engs):
                load_window(fam, p, 0, XCUT, engs=engs)

            for grp in range(NPAIRS):
                dps = [2 * grp, 2 * grp + 1]
                if grp > 0:
                    ensure_pair(grp + 3)

                ot = opool.tile([128, PLANE_OUT], MM_DT, tag="ot")
                for cidx, (c0, csz) in enumerate(CHUNKS):
                    pss = [ppool.tile([128, 512], F32, tag="ps", name="ps")
                           for _ in dps]
                    # j 0-8 fused kd01 (c128, W[dp], off kh*48+kw);
                    # j 9-11 fused kd2 kh01 (c128, W2[dp+2], off kw);
                    # j 12-14 kd2 kh2 (c64, W2[dp+2] lower, off 96+kw).
                    # Same-contract matmuls contiguous; serpentine the
                    # kind order across chunks so chunk boundaries don't
                    # add a contract-size switch.
                    jorder = list(range(NWBLK))
                    if cidx % 2 == 1:
                        jorder = jorder[13:] + jorder[:13]
                    for jj, j in enumerate(jorder):
                        rows = 128 if j < 13 else 64
                        for ci in range(len(dps)):
                            dst = pss[ci][ci * 64:(ci + 1) * 64, 0:csz]
                            if j < 9:
                                kh, kw = divmod(j, 3)
                                win = windows[("w", dps[ci])]
                                off = kh * DHW + kw + c0
                            elif j < 12:
                                kw = j - 9
                                win = windows[("w2", dps[ci] + 2)]
                                off = kw + c0
                            elif j == 12:
                                win = windows[("w3", dps[ci] + 2)]
                                off = 2 * DHW + c0
                            else:
                                win = windows[("w2", dps[ci] + 2)]
                                off = 2 * DHW + 2 + c0
                            nc.tensor.matmul(
                                dst,
                                wtile[0:rows, j * 64:(j + 1) * 64],
                                win[0:rows, off:off + csz],
                                start=(jj == 0),
                                stop=(jj == NWBLK - 1),
                            )
                    for ci in range(len(dps)):
                        nc.scalar.activation(
                            ot[ci * 64:(ci + 1) * 64, c0:c0 + csz],
                            pss[ci][ci * 64:(ci + 1) * 64, 0:csz],
                            mybir.ActivationFunctionType.Identity,
                            bias=btile[ci * 64:(ci + 1) * 64, :],
                        )
                    if grp == NPAIRS - 1:
                        # last pair: per-chunk store on the idle GpSimd
                        # queue so the final transfer overlaps the
                        # remaining chunks' matmuls
                        nc.gpsimd.dma_start(
                            out=y[grp, :, c0:c0 + csz],
                            in_=ot[:, c0:c0 + csz])
                    if grp == 0:
                        # staged loads: anything issued before the first
                        # matmuls inflates their DMA-wait threshold
                        if cidx == 0:
                            for fam, p in p0wins:
                                load_window(fam, p, XCUT, WCOLS)
                            ensure_pair(1)
                        elif cidx == 1:
                            ensure_pair(2)
                        elif cidx == 2:
                            ensure_pair(3)
                if grp < NPAIRS - 1:
                    nc.gpsimd.dma_start(out=y[grp, :, :], in_=ot[:, :])
    nc.compile()
    return nc


def _prep_in_maps(x, s, style_weight, style_bias, weight, bias):
    style = s @ style_weight.T + style_bias                      # [N, Cin]
    wm = weight[None] * style[:, None, :, None, None, None]      # [N,Co,Ci,k,k,k]
    wm = wm * (1.0 / np.sqrt((wm * wm).sum(axis=(2, 3, 4, 5), keepdims=True) + EPS))
    wk = wm.transpose(0, 2, 3, 4, 5, 1)                          # [N,Ci,kd,kh,kw,Co]
    wfull = np.zeros((N, 128, NWBLK * COUT), np.float32)
    for j in range(9):
        kh, kw = divmod(j, 3)
        wfull[:, 0:64, j * 64:(j + 1) * 64] = wk[:, :, 0, kh, kw, :]
        wfull[:, 64:128, j * 64:(j + 1) * 64] = wk[:, :, 1, kh, kw, :]
    for kw in range(3):
        j = 9 + kw
        wfull[:, 0:64, j * 64:(j + 1) * 64] = wk[:, :, 2, 0, kw, :]
        wfull[:, 64:128, j * 64:(j + 1) * 64] = wk[:, :, 2, 1, kw, :]
    # j12: (kd2,kh2,kw0)+(kd2,kh2,kw1) fused on the +1-col-shift window
    wfull[:, 0:64, 12 * 64:13 * 64] = wk[:, :, 2, 2, 0, :]
    wfull[:, 64:128, 12 * 64:13 * 64] = wk[:, :, 2, 2, 1, :]
    # j13: (kd2,kh2,kw2) alone, contract-64
    wfull[:, 0:64, 13 * 64:14 * 64] = wk[:, :, 2, 2, 2, :]
    wfull = np.ascontiguousarray(wfull.astype(NP_MM))
    bt = np.ascontiguousarray(
        np.tile(bias[:, None], (2, 1)), dtype=np.float32)        # [128,1]

    in_maps = []
    for core in range(NCORES):
        n, h = divmod(core, 2)
        h0 = h * H_HALF
        xsl = x[n, :, :, h0:h0 + H_IN, :].reshape(CIN, DHW * PLANE)
        xsl = np.concatenate(
            [xsl, np.zeros((CIN, PAD_COLS), np.float32)], axis=1)
        in_maps.append({
            "xs": np.ascontiguousarray(xsl.astype(NP_MM)),
            "wt": wfull[n],
            "bt": bt,
        })
    return in_maps


def _gather(results):
    y = np.empty((N, COUT, DOUT, DOUT, DOUT), np.float32)
    for core in range(NCORES):
        n, h = divmod(core, 2)
        planes = results[core]["y"].astype(np.float32).reshape(
            NPAIRS, 2, COUT, H_HALF, DHW)       # [pair, ci, co, h', w]
        full = planes.transpose(2, 0, 1, 3, 4).reshape(
            COUT, DOUT, H_HALF, DHW)            # [co, d'=2p+ci, h', w]
        y[n, :, :, h * H_HALF:(h + 1) * H_HALF, :] = full[:, :, :, :DOUT]
    return y


def kernel(x, s, style_weight, style_bias, weight, bias):
    global LAST_RESULTS
    x = np.asarray(x, np.float32)
    s = np.asarray(s, np.float32)
    style_weight = np.asarray(style_weight, np.float32)
    style_bias = np.asarray(style_bias, np.float32)
    weight = np.asarray(weight, np.float32)
    bias = np.asarray(bias, np.float32)

    if "nc" not in _CACHE:
        _CACHE["nc"] = _build_bass()
    in_maps = _prep_in_maps(x, s, style_weight, style_bias, weight, bias)
    res = None
    for attempt in range(3):
        try:
            res = run_bass_kernel_spmd(_CACHE["nc"], in_maps, list(range(NCORES)))
            break
        except Exception:
            if attempt == 2:
                raise
            time.sleep(30)  # transient device wedge; recovers on its own
    LAST_RESULTS = res
    return _gather(res.results)


# revision 38
# speedup vs baseline: 1.2535x; 1.2535x over previous
"""ConvMod3d (StyleGAN-style modulated 3x3x3 conv, N=4 groups) on 8 trn2 cores.

Sharding: 8 shards = 4 samples x 2 H-halves (was 2 D-halves). Each core
convolves 48 full-depth slabs of 25 h-rows (64ch x 25x48) against its
sample's modulated 64x64x27 weights, producing all 46 output D-planes x 23
h'-rows. 46 planes = 23 EVEN pairs, so the PE col-strip pairing never
degrades to a half-rate single-plane group (the D-split's odd 23rd plane
cost a full-rate group span). Style modulation/demodulation on host; the
conv (99.8% of FLOPs) on device.

Per output plane d': 27 taps, each a [Cin=64 -> Cout=64] matmul over the
flattened (h,w) slab (25x48 -> window of 1216 cols) with a shifted read
offset; invalid edge columns (w'>=46) are computed and discarded on host.

PE packing (trn2 constraints: row tiling crashes the device; alternating
contract sizes back-to-back costs 2.2x). Taps packed two-per-matmul on the
contraction dim via stacked SBUF windows:
- W[d]  = slab d (partitions 0-63) | slab d+1 (64-127): fuses the
  (kd=0,kd=1) tap pairs -> 9 contract-128 streams per output plane.
- W2[d] = slab d | slab d shifted +48 cols (one h row): fuses the
  (kd=2, kh=0/1) pairs -> 3 contract-128 streams; the 3 (kd=2,kh=2)
  taps stay contract-64 on W2's lower half.
Two output planes run concurrently on PE col strips. Matmuls in bf16
(fp32 PSUM accumulation).

Head: only the first pair's low window columns are DMA'd before the first
matmuls (a matmul's DMA wait is a coarse semaphore threshold covering
every DMA issued before it); the rest stream in behind pair-0's chunks.
"""

import time

import numpy as np
import ml_dtypes

import concourse.bacc as bacc
import concourse.bass as bass
import concourse.tile as tile
from concourse import mybir
from concourse.bass_utils import run_bass_kernel_spmd

EPS = 1e-8
N, CIN, COUT = 4, 64, 64
DHW, K = 48, 3
DOUT = DHW - K + 1          # 46 output planes per core (full depth)
H_HALF = DOUT // 2          # 23 output h'-rows per core
H_IN = H_HALF + K - 1       # 25 input h-rows per core
PLANE = H_IN * DHW          # 1200 cols per slab (25 h-rows x 48 w)
PAD_COLS = 192              # tail slack so shifted reads stay in-bounds
XS_COLS = DHW * PLANE + PAD_COLS   # 48 slabs
WCOLS = 1216                # window columns; max offset used 98+1103
PLANE_OUT = H_HALF * DHW    # 1104 computed output cols (23 h'-rows x 48)
NPAIRS = DOUT // 2          # 23 output-plane pairs
CHUNKS = [(0, 512), (512, 512), (1024, 80)]
NCORES = 8
NWBLK = 15                  # weight blocks of 64 cols
XCUT = 640                  # first-cut columns (covers chunk 0: off<=610)

F32 = mybir.dt.float32
MM_DT = mybir.dt.bfloat16
NP_MM = np.dtype(ml_dtypes.bfloat16)

_CACHE = {}
LAST_RESULTS = None  # BassKernelResults of the most recent device run


def _build_bass():
    nc = bacc.Bacc()
    xs = nc.declare_dram_parameter("xs", [CIN, XS_COLS], MM_DT, isOutput=False)
    wt = nc.declare_dram_parameter("wt", [128, NWBLK * COUT], MM_DT, isOutput=False)
    bt = nc.declare_dram_parameter("bt", [128, 1], F32, isOutput=False)
    # output stored bf16: halves output HBM traffic (measured DMA ceiling
    # is ~270GB/s and the f32 store pushed total demand right against it);
    # rounding adds ~0.1% rel err against a 2% budget
    y = nc.declare_dram_parameter(
        "y", [NPAIRS, 128, PLANE_OUT], MM_DT, isOutput=True)

    with tile.TileContext(nc) as tc:
        with (
            tc.tile_pool(name="const", bufs=1) as cpool,
            tc.tile_pool(name="xpool", bufs=16) as xpool,
            tc.tile_pool(name="opool", bufs=3) as opool,
            tc.tile_pool(name="ppool", bufs=8, space="PSUM") as ppool,
        ):
            wtile = cpool.tile([128, NWBLK * COUT], MM_DT)
            nc.sync.dma_start(out=wtile[:, :], in_=wt[:, :])
            btile = cpool.tile([128, 1], F32)
            nc.sync.dma_start(out=btile[:, :], in_=bt[:, :])

            windows = {}

            UPSHIFT = {"w": PLANE, "w2": DHW}

            def load_window(fam, p, a=0, b=WCOLS, engs=None):
                key = (fam, p)
                if key in windows:
                    xw = windows[key]
                else:
                    xw = xpool.tile([128, WCOLS], MM_DT, tag="xw", name="xw")
                    windows[key] = xw
                base = p * PLANE
                up = base + UPSHIFT[fam]
                e1, e2 = engs if engs else (nc.sync, nc.sync)
                e1.dma_start(out=xw[0:64, a:b],
                             in_=xs[:, base + a:base + b])
                e2.dma_start(out=xw[64:128, a:b],
                             in_=xs[:, up + a:up + b])

            def ensure_pair(p):
                if p >= NPAIRS:
                    return
                for d in (2 * p, 2 * p + 1):
                    if ("w", d) not in windows:
                        load_window("w", d)
                    if ("w2", d + 2) not in windows:
                        load_window("w2", d + 2)

            # PE warm-up: a few throwaway matmuls on the (first-loaded)
            # weight tile trip the HAM clock gate to K=8/8 while the first
            # windows stream in; without them the first ~5us of real
            # matmuls run at the 1.2GHz cold clock. Scratch PSUM, never
            # read.
            wps = ppool.tile([128, 512], F32, tag="ps", name="ps")
            for _ in range(10):
                nc.tensor.matmul(
                    wps[0:64, 0:512],
                    wtile[0:128, 0:64],
                    wtile[0:128, 0:512],
                    start=True, stop=True,
                )

            # pair 0: low columns only before the first matmuls, issue
            # spread across the three DMA-capable queues — each
            # DMA_DIRECT2D issue costs ~600ns and the queues run them
            # serially, so single-queue issue alone costs ~5us of head
            p0wins = [("w", 0), ("w", 1), ("w2", 2), ("w2", 3)]
            issue_engs = [(nc.scalar, nc.gpsimd), (nc.sync, nc.scalar),
                          (nc.gpsimd, nc.sync), (nc.scalar, nc.gpsimd)]
            for (fam, p), engs in zip(p0wins, issue_engs):
                load_window(fam, p, 0, XCUT, engs=engs)

            for grp in range(NPAIRS):
                dps = [2 * grp, 2 * grp + 1]
                if grp > 0:
                    ensure_pair(grp + 3)

                ot = opool.tile([128, PLANE_OUT], MM_DT, tag="ot")
                for cidx, (c0, csz) in enumerate(CHUNKS):
                    pss = [ppool.tile([128, 512], F32, tag="ps", name="ps")
                           for _ in dps]
                    # j 0-8 fused kd01 (c128, W[dp], off kh*48+kw);
                    # j 9-11 fused kd2 kh01 (c128, W2[dp+2], off kw);
                    # j 12-14 kd2 kh2 (c64, W2[dp+2] lower, off 96+kw).
                    # Same-contract matmuls contiguous; serpentine the
                    # kind order across chunks so chunk boundaries don't
                    # add a contract-size switch.
                    jorder = list(range(NWBLK))
                    if cidx % 2 == 1:
                        jorder = jorder[12:] + jorder[:12]
                    for jj, j in enumerate(jorder):
                        rows = 128 if j < 12 else 64
                        for ci in range(len(dps)):
                            dst = pss[ci][ci * 64:(ci + 1) * 64, 0:csz]
                            if j < 9:
                                kh, kw = divmod(j, 3)
                                win = windows[("w", dps[ci])]
                                off = kh * DHW + kw + c0
                            elif j < 12:
                                kw = j - 9
                                win = windows[("w2", dps[ci] + 2)]
                                off = kw + c0
                            else:
                                kw = j - 12
                                win = windows[("w2", dps[ci] + 2)]
                                off = 2 * DHW + kw + c0
                            nc.tensor.matmul(
                                dst,
                                wtile[0:rows, j * 64:(j + 1) * 64],
                                win[0:rows, off:off + csz],
                                start=(jj == 0),
                                stop=(jj == NWBLK - 1),
                            )
                    for ci in range(len(dps)):
                        nc.scalar.activation(
                            ot[ci * 64:(ci + 1) * 64, c0:c0 + csz],
                            pss[ci][ci * 64:(ci + 1) * 64, 0:csz],
                            mybir.ActivationFunctionType.Identity,
                            bias=btile[ci * 64:(ci + 1) * 64, :],
                        )
                    if grp == NPAIRS - 1:
                        # last pair: per-chunk store on the idle GpSimd
                        # queue so the final transfer overlaps the
                        # remaining chunks' matmuls
                        nc.gpsimd.dma_start(
                            out=y[grp, :, c0:c0 + csz],
                            in_=ot[:, c0:c0 + csz])
                    if grp == 0:
                        # staged loads: anything issued before the first
                        # matmuls inflates their DMA-wait threshold
                        if cidx == 0:
                            for fam, p in p0wins:
                                load_window(fam, p, XCUT, WCOLS)
                            ensure_pair(1)
                        elif cidx == 1:
                            ensure_pair(2)
                        elif cidx == 2:
                            ensure_pair(3)
                if grp < NPAIRS - 1:
                    nc.gpsimd.dma_start(out=y[grp, :, :], in_=ot[:, :])
    nc.compile()
    return nc


def _prep_in_maps(x, s, style_weight, style_bias, weight, bias):
    style = s @ style_weight.T + style_bias                      # [N, Cin]
    wm = weight[None] * style[:, None, :, None, None, None]      # [N,Co,Ci,k,k,k]
    wm = wm * (1.0 / np.sqrt((wm * wm).sum(axis=(2, 3, 4, 5), keepdims=True) + EPS))
    wk = wm.transpose(0, 2, 3, 4, 5, 1)                          # [N,Ci,kd,kh,kw,Co]
    wfull = np.zeros((N, 128, NWBLK * COUT), np.float32)
    for j in range(9):
        kh, kw = divmod(j, 3)
        wfull[:, 0:64, j * 64:(j + 1) * 64] = wk[:, :, 0, kh, kw, :]
        wfull[:, 64:128, j * 64:(j + 1) * 64] = wk[:, :, 1, kh, kw, :]
    for kw in range(3):
        j = 9 + kw
        wfull[:, 0:64, j * 64:(j + 1) * 64] = wk[:, :, 2, 0, kw, :]
        wfull[:, 64:128, j * 64:(j + 1) * 64] = wk[:, :, 2, 1, kw, :]
    for kw in range(3):
        j = 12 + kw
        wfull[:, 0:64, j * 64:(j + 1) * 64] = wk[:, :, 2, 2, kw, :]
    wfull = np.ascontiguousarray(wfull.astype(NP_MM))
    bt = np.ascontiguousarray(
        np.tile(bias[:, None], (2, 1)), dtype=np.float32)        # [128,1]

    in_maps = []
    for core in range(NCORES):
        n, h = divmod(core, 2)
        h0 = h * H_HALF
        xsl = x[n, :, :, h0:h0 + H_IN, :].reshape(CIN, DHW * PLANE)
        xsl = np.concatenate(
            [xsl, np.zeros((CIN, PAD_COLS), np.float32)], axis=1)
        in_maps.append({
            "xs": np.ascontiguousarray(xsl.astype(NP_MM)),
            "wt": wfull[n],
            "bt": bt,
        })
    return in_maps


def _gather(results):
    y = np.empty((N, COUT, DOUT, DOUT, DOUT), np.float32)
    for core in range(NCORES):
        n, h = divmod(core, 2)
        planes = results[core]["y"].astype(np.float32).reshape(
            NPAIRS, 2, COUT, H_HALF, DHW)       # [pair, ci, co, h', w]
        full = planes.transpose(2, 0, 1, 3, 4).reshape(
            COUT, DOUT, H_HALF, DHW)            # [co, d'=2p+ci, h', w]
        y[n, :, :, h * H_HALF:(h + 1) * H_HALF, :] = full[:, :, :, :DOUT]
    return y


def kernel(x, s, style_weight, style_bias, weight, bias):
    global LAST_RESULTS
    x = np.asarray(x, np.float32)
    s = np.asarray(s, np.float32)
    style_weight = np.asarray(style_weight, np.float32)
    style_bias = np.asarray(style_bias, np.float32)
    weight = np.asarray(weight, np.float32)
    bias = np.asarray(bias, np.float32)

    if "nc" not in _CACHE:
        _CACHE["nc"] = _build_bass()
    in_maps = _prep_in_maps(x, s, style_weight, style_bias, weight, bias)
    res = None
    for attempt in range(3):
        try:
            res = run_bass_kernel_spmd(_CACHE["nc"], in_maps, list(range(NCORES)))
            break
        except Exception:
            if attempt == 2:
                raise
            time.sleep(30)  # transient device wedge; recovers on its own
    LAST_RESULTS = res
    return _gather(res.results)
